# revision 53
# baseline (speedup 1.0000x reference)
"""Sparse (segment + causal) GQA attention on 8 Trainium2 NeuronCores.

Problem: nn_AttentionOp_27719718928719
  query (2, 1024, 32, 128) f32, key/value (2, 1024, 8, 128) f32,
  decoder_segment_ids (2, 1024) i32 (sorted) -> out (2, 1024, 32, 128) f32

Sharding: core c owns kv-head c and its 4 GQA query heads, both batches.
Perfect shard (no replication): Q, K, V, and the output all split 8 ways, and
the compiled program is identical on every core (the block schedule depends
only on the segment ids, which all cores share).

Device algorithm, one unit per (batch b, 128-query block tj) with all 4 heads
fused along the free axis (up to 512 wide), restricted per key-block to the
live (unmasked) t-column span:
  for each valid key block si (causal + segment overlap, host-computed):
    S^T[s, (h,t)] = K[si]^T Q      1 fp16 matmul over live cols only
    P^T = exp(S^T + bias_cls[s])   ACT; the segment row-mask is folded into
                                   the per-partition exp bias (-30 normal,
                                   -60030 for masked key rows); two ACT
                                   instructions for boundary-diag blocks
                                   whose mask class changes mid-block
    causal zero (diag blocks only) one DVE tri-mask multiply, all 4 heads
    outT[d, (h,t)] += V[si]' P^T   bf16 matmul over live cols
    sums[tj, (h,t)] += 1' P^T      bf16 one-hot matmul into a shared
                                   per-batch PSUM tile (row tj); the tile is
                                   zeroed once by DVE so every matmul
                                   accumulates (start=False)
  stage outT PSUM -> bf16 SBUF (DVE; bf16 for range -- the unnormalized
  accumulators overflow fp16), DMA out in tj pairs; sums DMA'd straight
  from PSUM per batch.
No softmax max-subtraction: logits are O(+-50) so exp(x-30) stays in
fp32/bf16 range and exp/sum(exp) matches the reference exactly.
Host does the (cheap) normalization out/sums and all layout transposes.

PE work is software-pipelined: each PSUM-pair's QK matmuls are emitted before
the previous pair's PV/sums so the Tensor engine never head-of-line blocks on
the ACT exp. Input DMAs are ordered so the first unit's Q/K/V blocks land
first.
"""

import numpy as np
import ml_dtypes

B, T, S, NQ, NKV, D = 2, 1024, 1024, 32, 8, 128
G = NQ // NKV
BLK = 128
NBLK = S // BLK  # 8
W = G * BLK  # 512: fused 4-head free width
N_CORES = 8
HLOC = NQ // N_CORES  # 4
MASK_BIAS = -60000.0
EXP_SHIFT = -30.0

_compiled_cache = {}

# Test-only knobs (the grading path never sets these): when TRACE is true the
# SPMD run captures an NTFF profile into TRACE_DIR.
TRACE = False
TRACE_DIR = None


def _segment_structure(seg):
    """Block schedule for one batch's (sorted) segment ids.

    Returns (units, classes):
      units[tj] = list of block dicts:
        si     key block index
        a, e   live t-column span (outside it every key row of si is masked)
        spans  [(a, e, cls)]: exp bias class per t-span covering [a, e);
               cls 0 = causal-only (no row mask), cls >= 1 indexes classes
        diag   causal in-block mask needed
      classes = list of np bool arrays [BLK] (True = masked key row) for
        cls >= 1 (classes[0] unused placeholder).
    """
    seg = np.asarray(seg)
    t_idx = np.arange(S)
    seg_start = np.zeros(S, np.int64)
    seg_end = np.zeros(S, np.int64)
    for v in np.unique(seg):
        m = seg == v
        lo, hi = np.argmax(m), S - np.argmax(m[::-1])
        seg_start[m], seg_end[m] = lo, hi
    valid_ts = (t_idx[None, :] <= t_idx[:, None]) & (seg[None, :] == seg[:, None])
    v4 = valid_ts.reshape(NBLK, BLK, NBLK, BLK)
    vblk = v4.any(axis=(1, 3))  # [tj, si]
    fblk = v4.all(axis=(1, 3))

    classes = [np.zeros(BLK, dtype=bool)]  # cls 0 = no row mask
    cls_key = {classes[0].tobytes(): 0}

    def class_id(mask_rows):
        key = mask_rows.tobytes()
        if key not in cls_key:
            cls_key[key] = len(classes)
            classes.append(mask_rows.copy())
        return cls_key[key]

    units = []
    for tj in range(NBLK):
        blocks = []
        sis = [si for si in range(NBLK) if vblk[tj, si]]
        assert sis == list(range(min(sis), max(sis) + 1))
        for si in sis:
            diag = si == tj
            spans = []  # (a, e, cls) over live cols
            if fblk[tj, si] or (diag and _only_causal(v4, tj, si)):
                spans.append((0, BLK, 0))
            else:
                tcols = np.arange(tj * BLK, (tj + 1) * BLK)
                lo_rel = np.clip(seg_start[tcols] - si * BLK, 0, BLK)
                hi_rel = np.clip(seg_end[tcols] - si * BLK, 0, BLK)
                a = 0
                for i in range(1, BLK + 1):
                    if i == BLK or lo_rel[i] != lo_rel[a] or hi_rel[i] != hi_rel[a]:
                        lo, hi = int(lo_rel[a]), int(hi_rel[a])
                        rows = np.ones(BLK, dtype=bool)
                        rows[lo:hi] = False  # False -> keep
                        if not rows.all():  # live span (dead spans skipped)
                            spans.append((a, i, class_id(rows)))
                        a = i
            a0, e0 = spans[0][0], spans[-1][1]
            assert [s[0] for s in spans[1:]] == [s[1] for s in spans[:-1]], (
                "live span must be contiguous"
            )
            blocks.append(dict(si=si, a=a0, e=e0, spans=spans, diag=diag))
        # order blocks to let equal-(span, cls) neighbors share one ACT exp:
        # stable-sort single-span blocks by key, multi-span blocks last
        blocks.sort(key=lambda blk: (len(blk["spans"]) > 1, blk["spans"][0]))
        units.append(blocks)
    return units, classes


def _only_causal(v4, tj, si):
    """True if block (tj, si)'s invalid entries are exactly the causal ones."""
    blk = v4[tj, :, si, :]  # [t, s]
    t = np.arange(BLK)[:, None] + tj * BLK
    s = np.arange(BLK)[None, :] + si * BLK
    return bool((blk == (s <= t)).all())


def _build_program(all_units, all_classes):
    """Build the SPMD Bass program. all_units/all_classes indexed by batch."""
    import concourse.bass as bass  # noqa: F401
    from concourse import bacc
    import concourse.mybir as mybir
    import concourse.tile as tile

    DT = mybir.dt
    F16 = DT.float16
    BF16 = DT.bfloat16
    F32 = DT.float32
    ncls = [len(c) for c in all_classes]
    nc = bacc.Bacc(None, target_bir_lowering=False, debug=False)

    q_d = nc.dram_tensor("q", [B, D, NBLK, HLOC, BLK], F16, kind="ExternalInput").ap()
    k_d = nc.dram_tensor("k", [B, D, S], F16, kind="ExternalInput").ap()
    v_d = nc.dram_tensor("v", [B, NBLK, BLK, D], BF16, kind="ExternalInput").ap()
    # exp bias vectors: col (cls_base[b] + cls) = -30 plus -60000 on masked rows
    nbias = ncls[0] + ncls[1]
    ebias_d = nc.dram_tensor("ebias", [BLK, nbias], F32, kind="ExternalInput").ap()
    # onehot[:, tj, r] = (r == tj): sums weights routing unit tj's softmax
    # denominators into row tj of the shared per-batch PSUM tile (PE output
    # base partition must be 0/32/64, so separate [1, W] rows aren't usable)
    onehot_d = nc.dram_tensor(
        "onehot", [BLK, NBLK, NBLK], BF16, kind="ExternalInput"
    ).ap()
    # tri[s, h, t] = (s <= t): causal keep-mask for diag blocks, applied as a
    # bf16 multiply on DVE (all-16-bit operands hit the fast DVE path)
    tri_d = nc.dram_tensor("tri", [BLK, HLOC, BLK], BF16, kind="ExternalInput").ap()
    outT_d = nc.dram_tensor("outT", [B, D, NBLK, HLOC, BLK], BF16,
                            kind="ExternalOutput").ap()
    sums_d = nc.dram_tensor("sums", [B, NBLK, HLOC, BLK], F32,
                            kind="ExternalOutput").ap()

    cls_base = [0, ncls[0]]  # class index offset per batch

    # unit emission order: b1 runs its lightest unit (tj0: diag only) last so
    # the final pipeline drain is as short as possible; b0 keeps ascending
    # order to match the input DMA arrival order.
    unit_order = [(0, tj) for tj in range(NBLK)] + \
                 [(1, tj) for tj in list(range(1, NBLK)) + [0]]
    # output DMA groups: consecutive-tj unit pairs share one staging tile+DMA
    out_groups = [[(0, 0), (0, 1)], [(0, 2), (0, 3)], [(0, 4), (0, 5)],
                  [(0, 6), (0, 7)], [(1, 1), (1, 2)], [(1, 3), (1, 4)],
                  [(1, 5), (1, 6)], [(1, 7)], [(1, 0)]]
    unit_group = {}
    for g in out_groups:
        for slot, u in enumerate(g):
            unit_group[u] = (g, slot)

    with tile.TileContext(nc) as tc:
        with (
            tc.tile_pool(name="const", bufs=1) as constp,
            tc.tile_pool(name="qkv", bufs=1) as qkv,
            tc.tile_pool(name="pt", bufs=6) as ptp,
            tc.tile_pool(name="stage", bufs=3) as stagep,
            tc.tile_pool(name="sstage", bufs=2) as sstagep,
            tc.tile_pool(name="ptsum", bufs=5) as ptsp,
            tc.tile_pool(name="ps_s", bufs=2, space="PSUM") as ps_s,
            tc.tile_pool(name="ps_o", bufs=3, space="PSUM") as ps_o,
            tc.tile_pool(name="ps_m", bufs=1, space="PSUM") as ps_m,
        ):
            k_t = qkv.tile([D, B, S], F16)
            q_t = qkv.tile([D, B, NBLK, HLOC, BLK], F16)
            v_t = qkv.tile([BLK, B, NBLK, D], BF16)
            onehot_t = constp.tile([BLK, NBLK, NBLK], BF16)
            tri_t = constp.tile([BLK, HLOC, BLK], BF16)
            ebias_t = constp.tile([BLK, nbias], F32)

            # Input DMAs in per-tj consumption order, issue-parallelized
            # across four engine queues so the early units' inputs land first.
            nc.sync.dma_start(out=q_t[:, 0, 0], in_=q_d[0, :, 0])
            nc.gpsimd.dma_start(out=k_t[:, 0, 0:2 * BLK], in_=k_d[0, :, 0:2 * BLK])
            nc.scalar.dma_start(
                out=v_t[:, 0, 0:2],
                in_=v_d[0, 0:2].rearrange("si p d -> p si d"),
            )
            nc.sync.dma_start(out=q_t[:, 0, 1], in_=q_d[0, :, 1])
            nc.gpsimd.dma_start(out=onehot_t, in_=onehot_d)
            nc.scalar.dma_start(out=ebias_t, in_=ebias_d)
            nc.sync.dma_start(out=q_t[:, 0, 2:4], in_=q_d[0, :, 2:4])
            nc.gpsimd.dma_start(out=k_t[:, 0, 2 * BLK:4 * BLK],
                                in_=k_d[0, :, 2 * BLK:4 * BLK])
            nc.scalar.dma_start(out=tri_t, in_=tri_d)
            nc.sync.dma_start(
                out=v_t[:, 0, 2:4],
                in_=v_d[0, 2:4].rearrange("si p d -> p si d"),
            )
            nc.gpsimd.dma_start(out=k_t[:, 0, 4 * BLK:], in_=k_d[0, :, 4 * BLK:])
            nc.sync.dma_start(out=q_t[:, 0, 4:], in_=q_d[0, :, 4:])
            nc.gpsimd.dma_start(
                out=v_t[:, 0, 4:],
                in_=v_d[0, 4:].rearrange("si p d -> p si d"),
            )
            nc.gpsimd.dma_start(out=k_t[:, 1], in_=k_d[1])
            nc.gpsimd.dma_start(
                out=v_t[:, 1], in_=v_d[1].rearrange("si p d -> p si d")
            )
            nc.sync.dma_start(out=q_t[:, 1, 0:4], in_=q_d[1, :, 0:4])
            nc.sync.dma_start(out=q_t[:, 1, 4:], in_=q_d[1, :, 4:])

            # Build the flat pair-task list (software pipeline over PE).
            tasks = []
            for b, tj in unit_order:
                blocks = all_units[b][tj]
                unit = {"b": b, "tj": tj, "n_blk": len(blocks)}
                # per-col first/last live si in execution order, for PV/sums
                # start/stop flags (cols of a later block may start fresh)
                first_of = {}
                last_of = {}
                for bi, blk in enumerate(blocks):
                    for col in range(blk["a"], blk["e"]):
                        if col not in first_of:
                            first_of[col] = bi
                        last_of[col] = bi
                for bi, blk in enumerate(blocks):
                    # split [a, e) into runs of constant (start, stop)
                    ops = []
                    ra = blk["a"]
                    rflags = None
                    for col in range(blk["a"], blk["e"] + 1):
                        flags = (first_of[col] == bi, last_of[col] == bi) \
                            if col < blk["e"] else None
                        if flags != rflags:
                            if rflags is not None and col > ra:
                                ops.append((ra, col, rflags[0], rflags[1]))
                            ra = col
                            rflags = flags
                    blk["pv_ops"] = ops
                pairs = [blocks[i:i + 2] for i in range(0, len(blocks), 2)]
                for pi, pair in enumerate(pairs):
                    tasks.append({
                        "unit": unit, "pair": pair,
                        "first": pi == 0, "last": pi == len(pairs) - 1,
                    })

            # Per-batch shared sums PSUM tile: row tj accumulates unit tj's
            # softmax denominators; DVE zeroes it once, every sums matmul
            # accumulates with start=False, and the tile is DMA'd straight
            # from PSUM at batch end.
            sm_all = [None, None]
            group_stage = {}
            dma_rr = [0]  # round-robin out-DMA issuing engine

            def emit_qk(task):
                unit = task["unit"]
                b, tj = unit["b"], unit["tj"]
                if task["first"]:
                    unit["outp"] = ps_o.tile([D, HLOC, BLK], F32, name="outp")
                    if sm_all[b] is None:
                        sm_all[b] = ps_m.tile([NBLK, HLOC, BLK], F32,
                                              name="sm_all")
                        nc.vector.memset(sm_all[b], 0.0)
                pair = task["pair"]
                st = ps_s.tile([BLK, 2, HLOC, BLK], F32, name="st")
                pt = ptp.tile([BLK, 2, HLOC, BLK], BF16, name="pt")
                for j, blk in enumerate(pair):
                    kh = k_t[:, b, blk["si"] * BLK:(blk["si"] + 1) * BLK]
                    nc.tensor.matmul(st[:, j, :, blk["a"]:blk["e"]],
                                     kh, q_t[:, b, tj, :, blk["a"]:blk["e"]],
                                     start=True, stop=True,
                                     skip_group_check=True)
                # one ACT exp per (span, cls) run; a clean pair fuses into one
                runs = []  # (j0, j1, a, e, cls) over pair halves
                for j, blk in enumerate(pair):
                    for a, e, cls in blk["spans"]:
                        cid = 0 if cls == 0 else cls_base[b] + cls
                        if (runs and runs[-1][2] == a and runs[-1][3] == e
                                and runs[-1][4] == cid
                                and runs[-1][1] == j):
                            runs[-1] = (runs[-1][0], j + 1, a, e, cid)
                        else:
                            runs.append((j, j + 1, a, e, cid))
                for j0, j1, a, e, cid in runs:
                    nc.scalar.activation(
                        out=pt[:, j0:j1, :, a:e], in_=st[:, j0:j1, :, a:e],
                        func=mybir.ActivationFunctionType.Exp,
                        bias=ebias_t[:, cid:cid + 1],
                    )
                for j, blk in enumerate(pair):
                    if blk["diag"]:
                        # zero s > t for every head via the tri keep-mask
                        nc.vector.tensor_mul(
                            out=pt[:, j], in0=pt[:, j], in1=tri_t
                        )
                task["pt"] = pt
                # pre-add equal-span pair halves on DVE now (bf16 fast path),
                # a unit ahead of the sums matmul that consumes the result —
                # the unit of pipeline slack hides the add's latency
                if (len(pair) == 2 and pair[0]["a"] == pair[1]["a"]
                        and pair[0]["e"] == pair[1]["e"]):
                    a, e = pair[0]["a"], pair[0]["e"]
                    ps = ptsp.tile([BLK, HLOC, BLK], BF16, name="ptsum")
                    nc.vector.tensor_add(
                        out=ps[:, :, a:e],
                        in0=pt[:, 0, :, a:e], in1=pt[:, 1, :, a:e],
                    )
                    task["ptsum"] = (ps, a, e)
                else:
                    task["ptsum"] = None

            def emit_pv_unit(utasks):
                unit = utasks[0]["unit"]
                b, tj = unit["b"], unit["tj"]
                batch_last_unit = (b, tj) in ((0, 7), (1, 0))
                # all sums matmuls of the unit back-to-back first: they share
                # the one-hot weights (skips most of the LDWEIGHTS cost), and
                # the sums staging then overlaps the PV/CAST chain. One
                # matmul per block over its full live span (sums only
                # accumulate, so the pv_op start/stop splits don't apply).
                sum_ops = []
                for task in utasks:
                    if task["ptsum"] is not None:
                        ps, a, e = task["ptsum"]
                        sum_ops.append((ps, a, e))
                    else:
                        for j, blk in enumerate(task["pair"]):
                            sum_ops.append(
                                (task["pt"][:, j], blk["a"], blk["e"]))
                for oi, (src, a, e) in enumerate(sum_ops):
                    nc.tensor.matmul(
                        sm_all[b][:, :, a:e], onehot_t[:, tj],
                        src[:, :, a:e],
                        start=False,
                        stop=batch_last_unit and oi == len(sum_ops) - 1,
                        skip_group_check=True,
                    )
                for task in utasks:
                    for j, blk in enumerate(task["pair"]):
                        si = blk["si"]
                        for a, e, fst, lst in blk["pv_ops"]:
                            nc.tensor.matmul(
                                unit["outp"][:, :, a:e], v_t[:, b, si],
                                task["pt"][:, j, :, a:e],
                                start=fst, stop=lst, skip_group_check=True,
                            )
                if batch_last_unit:
                    s_sb = sstagep.tile([NBLK, HLOC, BLK], F32,
                                        name="sstage")
                    nc.vector.tensor_copy(out=s_sb, in_=sm_all[b])
                    nc.gpsimd.dma_start(out=sums_d[b], in_=s_sb)
                group, slot = unit_group[(b, tj)]
                gkey = id(group)
                if gkey not in group_stage:
                    group_stage[gkey] = stagep.tile(
                        [D, 2, HLOC, BLK], BF16, name="ostage"
                    )[:, :len(group)]
                o_sb = group_stage[gkey]
                nc.vector.tensor_copy(out=o_sb[:, slot], in_=unit["outp"])
                if slot == len(group) - 1:
                    tj0 = group[0][1]
                    eng = nc.sync if dma_rr[0] % 2 == 0 else nc.gpsimd
                    dma_rr[0] += 1
                    eng.dma_start(
                        out=outT_d[b, :, tj0:tj0 + len(group)], in_=o_sb
                    )

            # unit-level software pipeline with a 2-unit lookahead: PE always
            # has the next units' QK queued while earlier exps complete
            unit_seq = []
            for task in tasks:
                if task["first"]:
                    unit_seq.append([])
                unit_seq[-1].append(task)
            # flush-before-emit: the warmup queues THREE units of QK before
            # the first PV (hiding the first exp/tri chain), while the steady
            # state keeps the same 2-unit lookahead (PV(u) is always emitted
            # before QK(u+3) allocates its PSUM, so pool recycling resolves)
            window = []
            for utasks in unit_seq:
                if len(window) >= 3:
                    emit_pv_unit(window.pop(0))
                for task in utasks:
                    emit_qk(task)
                window.append(utasks)
            for utasks in window:
                emit_pv_unit(utasks)
    nc.compile()
    return nc


def kernel(query, key, value, decoder_segment_ids):
    from concourse.bass_utils import run_bass_kernel_spmd

    query = np.asarray(query, dtype=np.float32)
    key = np.asarray(key, dtype=np.float32)
    value = np.asarray(value, dtype=np.float32)
    seg = np.asarray(decoder_segment_ids, dtype=np.int32)

    structs = [_segment_structure(seg[b]) for b in range(B)]
    all_units = [s[0] for s in structs]
    all_classes = [s[1] for s in structs]
    sig = tuple(
        tuple(tuple((blk["si"], blk["a"], blk["e"], tuple(blk["spans"]),
                     blk["diag"]) for blk in blocks)
              for blocks in units)
        for units in all_units
    ) + tuple(c.tobytes() for cl in all_classes for c in cl)
    nc = _compiled_cache.get(sig)
    if nc is None:
        nc = _build_program(all_units, all_classes)
        _compiled_cache[sig] = nc

    onehot = np.zeros((BLK, NBLK, NBLK), dtype=ml_dtypes.bfloat16)
    for tj in range(NBLK):
        onehot[:, tj, tj] = 1.0
    s_i = np.arange(BLK)
    tri = (s_i[:, None] <= s_i[None, :]).astype(ml_dtypes.bfloat16)
    tri = np.ascontiguousarray(np.broadcast_to(tri[:, None, :], (BLK, HLOC, BLK)))

    nbias = sum(len(c) for c in all_classes)
    ebias = np.full((BLK, nbias), EXP_SHIFT, dtype=np.float32)
    i = 0
    for cl in all_classes:
        for rows in cl:
            ebias[:, i] += np.where(rows, MASK_BIAS, 0.0)
            i += 1

    in_maps = []
    for c in range(N_CORES):
        q_c = query[:, :, c * HLOC:(c + 1) * HLOC, :]  # (B, T, HLOC, D)
        # -> (B, D, NBLK, HLOC, BLK): element [b,d,tj,h,y] = q_c[b, tj*128+y, h, d]
        qT = np.ascontiguousarray(
            q_c.transpose(0, 3, 1, 2)  # (B, D, T, HLOC)
            .reshape(B, D, NBLK, BLK, HLOC)
            .transpose(0, 1, 2, 4, 3)
        ).astype(np.float16)
        kT = np.ascontiguousarray(
            key[:, :, c, :].transpose(0, 2, 1)
        ).astype(np.float16)  # (B, D, S)
        v_c = value[:, :, c, :].reshape(B, NBLK, BLK, D).astype(ml_dtypes.bfloat16)
        in_maps.append({"q": qT, "k": kT, "v": v_c, "ebias": ebias,
                        "onehot": onehot, "tri": tri})

    kwargs = {}
    if TRACE:
        kwargs = dict(trace=True, tmpdir=TRACE_DIR)
    res = run_bass_kernel_spmd(nc, in_maps, core_ids=list(range(N_CORES)), **kwargs)
    kernel.last_results = res

    out = np.empty((B, T, NQ, D), dtype=np.float32)
    for c in range(N_CORES):
        outT = res.results[c]["outT"]  # (B, D, NBLK, HLOC, BLK) bf16
        sums = res.results[c]["sums"]  # (B, NBLK, HLOC, BLK) f32
        o = outT.astype(np.float32).reshape(B, D, NBLK, HLOC, BLK)
        s = sums
        # out[b, tj*128+y, c*4+h, d] = o[b, d, tj, h, y] / s[b, tj, h, y]
        o = o.transpose(0, 2, 4, 3, 1).reshape(B, T, HLOC, D)
        s = s.transpose(0, 1, 3, 2).reshape(B, T, HLOC)
        out[:, :, c * HLOC:(c + 1) * HLOC, :] = o / s[:, :, :, None]
    return out


# revision 55
# speedup vs baseline: 1.1309x; 1.1309x over previous
"""Sparse (segment + causal) GQA attention on 8 Trainium2 NeuronCores.

Problem: nn_AttentionOp_27719718928719
  query (2, 1024, 32, 128) f32, key/value (2, 1024, 8, 128) f32,
  decoder_segment_ids (2, 1024) i32 (sorted) -> out (2, 1024, 32, 128) f32

Sharding: core c owns kv-head c and its 4 GQA query heads, both batches.
Perfect shard (no replication): Q, K, V, and the output all split 8 ways, and
the compiled program is identical on every core (the block schedule depends
only on the segment ids, which all cores share).

Device algorithm, one unit per (batch b, 128-query block tj) with all 4 heads
fused along the free axis (up to 512 wide), restricted per key-block to the
live (unmasked) t-column span:
  for each valid key block si (causal + segment overlap, host-computed):
    S^T[s, (h,t)] = K[si]^T Q      1 fp16 matmul over live cols only
    P^T = exp(S^T + bias_cls[s])   ACT; the segment row-mask is folded into
                                   the per-partition exp bias (-30 normal,
                                   -60030 for masked key rows); two ACT
                                   instructions for boundary-diag blocks
                                   whose mask class changes mid-block
    causal zero (diag blocks only) one DVE tri-mask multiply, all 4 heads
    outT[d, (h,t)] += V[si]' P^T   bf16 matmul over live cols
    sums[tj, (h,t)] += 1' P^T      bf16 one-hot matmul into a shared
                                   per-batch PSUM tile (row tj); the tile is
                                   zeroed once by DVE so every matmul
                                   accumulates (start=False)
  stage outT PSUM -> bf16 SBUF (DVE; bf16 for range -- the unnormalized
  accumulators overflow fp16), DMA out in tj pairs; sums DMA'd straight
  from PSUM per batch.
No softmax max-subtraction: logits are O(+-50) so exp(x-30) stays in
fp32/bf16 range and exp/sum(exp) matches the reference exactly.
Host does the (cheap) normalization out/sums and all layout transposes.

PE work is software-pipelined: each PSUM-pair's QK matmuls are emitted before
the previous pair's PV/sums so the Tensor engine never head-of-line blocks on
the ACT exp. Input DMAs are ordered so the first unit's Q/K/V blocks land
first.
"""

import numpy as np
import ml_dtypes

B, T, S, NQ, NKV, D = 2, 1024, 1024, 32, 8, 128
G = NQ // NKV
BLK = 128
NBLK = S // BLK  # 8
W = G * BLK  # 512: fused 4-head free width
N_CORES = 8
HLOC = NQ // N_CORES  # 4
MASK_BIAS = -60000.0
EXP_SHIFT = -30.0

_compiled_cache = {}

# Test-only knobs (the grading path never sets these): when TRACE is true the
# SPMD run captures an NTFF profile into TRACE_DIR.
TRACE = False
TRACE_DIR = None


def _segment_structure(seg):
    """Block schedule for one batch's (sorted) segment ids.

    Returns (units, classes):
      units[tj] = list of block dicts:
        si     key block index
        a, e   live t-column span (outside it every key row of si is masked)
        spans  [(a, e, cls)]: exp bias class per t-span covering [a, e);
               cls 0 = causal-only (no row mask), cls >= 1 indexes classes
        diag   causal in-block mask needed
      classes = list of np bool arrays [BLK] (True = masked key row) for
        cls >= 1 (classes[0] unused placeholder).
    """
    seg = np.asarray(seg)
    t_idx = np.arange(S)
    seg_start = np.zeros(S, np.int64)
    seg_end = np.zeros(S, np.int64)
    for v in np.unique(seg):
        m = seg == v
        lo, hi = np.argmax(m), S - np.argmax(m[::-1])
        seg_start[m], seg_end[m] = lo, hi
    valid_ts = (t_idx[None, :] <= t_idx[:, None]) & (seg[None, :] == seg[:, None])
    v4 = valid_ts.reshape(NBLK, BLK, NBLK, BLK)
    vblk = v4.any(axis=(1, 3))  # [tj, si]
    fblk = v4.all(axis=(1, 3))

    classes = [np.zeros(BLK, dtype=bool)]  # cls 0 = no row mask
    cls_key = {classes[0].tobytes(): 0}

    def class_id(mask_rows):
        key = mask_rows.tobytes()
        if key not in cls_key:
            cls_key[key] = len(classes)
            classes.append(mask_rows.copy())
        return cls_key[key]

    units = []
    for tj in range(NBLK):
        blocks = []
        sis = [si for si in range(NBLK) if vblk[tj, si]]
        assert sis == list(range(min(sis), max(sis) + 1))
        for si in sis:
            diag = si == tj
            spans = []  # (a, e, cls) over live cols
            if fblk[tj, si] or (diag and _only_causal(v4, tj, si)):
                spans.append((0, BLK, 0))
            else:
                tcols = np.arange(tj * BLK, (tj + 1) * BLK)
                lo_rel = np.clip(seg_start[tcols] - si * BLK, 0, BLK)
                hi_rel = np.clip(seg_end[tcols] - si * BLK, 0, BLK)
                a = 0
                for i in range(1, BLK + 1):
                    if i == BLK or lo_rel[i] != lo_rel[a] or hi_rel[i] != hi_rel[a]:
                        lo, hi = int(lo_rel[a]), int(hi_rel[a])
                        rows = np.ones(BLK, dtype=bool)
                        rows[lo:hi] = False  # False -> keep
                        if not rows.all():  # live span (dead spans skipped)
                            spans.append((a, i, class_id(rows)))
                        a = i
            a0, e0 = spans[0][0], spans[-1][1]
            assert [s[0] for s in spans[1:]] == [s[1] for s in spans[:-1]], (
                "live span must be contiguous"
            )
            blocks.append(dict(si=si, a=a0, e=e0, spans=spans, diag=diag))
        # order blocks to let equal-(span, cls) neighbors share one ACT exp:
        # stable-sort single-span blocks by key, multi-span blocks last
        blocks.sort(key=lambda blk: (len(blk["spans"]) > 1, blk["spans"][0]))
        units.append(blocks)
    return units, classes


def _only_causal(v4, tj, si):
    """True if block (tj, si)'s invalid entries are exactly the causal ones."""
    blk = v4[tj, :, si, :]  # [t, s]
    t = np.arange(BLK)[:, None] + tj * BLK
    s = np.arange(BLK)[None, :] + si * BLK
    return bool((blk == (s <= t)).all())


def _build_program(all_units, all_classes):
    """Build the SPMD Bass program. all_units/all_classes indexed by batch."""
    import concourse.bass as bass  # noqa: F401
    from concourse import bacc
    import concourse.mybir as mybir
    import concourse.tile as tile

    DT = mybir.dt
    F16 = DT.float16
    BF16 = DT.bfloat16
    F32 = DT.float32
    ncls = [len(c) for c in all_classes]
    nc = bacc.Bacc(None, target_bir_lowering=False, debug=False)

    q_d = nc.dram_tensor("q", [B, D, NBLK, HLOC, BLK], F16, kind="ExternalInput").ap()
    k_d = nc.dram_tensor("k", [B, D, S], F16, kind="ExternalInput").ap()
    v_d = nc.dram_tensor("v", [B, NBLK, BLK, D], BF16, kind="ExternalInput").ap()
    # exp bias vectors: col (cls_base[b] + cls) = -30 plus -60000 on masked rows
    nbias = ncls[0] + ncls[1]
    ebias_d = nc.dram_tensor("ebias", [BLK, nbias], F32, kind="ExternalInput").ap()
    # onehot[:, tj, r] = (r == tj): sums weights routing unit tj's softmax
    # denominators into row tj of the shared per-batch PSUM tile (PE output
    # base partition must be 0/32/64, so separate [1, W] rows aren't usable)
    onehot_d = nc.dram_tensor(
        "onehot", [BLK, NBLK, NBLK], BF16, kind="ExternalInput"
    ).ap()
    # tri[s, h, t] = (s <= t): causal keep-mask for diag blocks, applied as a
    # bf16 multiply on DVE (all-16-bit operands hit the fast DVE path)
    tri_d = nc.dram_tensor("tri", [BLK, HLOC, BLK], BF16, kind="ExternalInput").ap()
    outT_d = nc.dram_tensor("outT", [B, D, NBLK, HLOC, BLK], BF16,
                            kind="ExternalOutput").ap()
    sums_d = nc.dram_tensor("sums", [B, NBLK, HLOC, BLK], F32,
                            kind="ExternalOutput").ap()

    cls_base = [0, ncls[0]]  # class index offset per batch

    # unit emission order: b1 runs its lightest unit (tj0: diag only) last so
    # the final pipeline drain is as short as possible; b0 keeps ascending
    # order to match the input DMA arrival order.
    unit_order = [(0, tj) for tj in range(NBLK)] + \
                 [(1, tj) for tj in list(range(1, NBLK)) + [0]]
    # output DMA groups: consecutive-tj unit pairs share one staging tile+DMA
    out_groups = [[(0, 0), (0, 1)], [(0, 2), (0, 3)], [(0, 4), (0, 5)],
                  [(0, 6), (0, 7)], [(1, 1), (1, 2)], [(1, 3), (1, 4)],
                  [(1, 5), (1, 6)], [(1, 7)], [(1, 0)]]
    unit_group = {}
    for g in out_groups:
        for slot, u in enumerate(g):
            unit_group[u] = (g, slot)

    with tile.TileContext(nc) as tc:
        with (
            tc.tile_pool(name="const", bufs=1) as constp,
            tc.tile_pool(name="qkv", bufs=1) as qkv,
            tc.tile_pool(name="pt", bufs=6) as ptp,
            tc.tile_pool(name="stage", bufs=3) as stagep,
            tc.tile_pool(name="sstage", bufs=2) as sstagep,
            tc.tile_pool(name="ptsum", bufs=4) as ptsp,
            tc.tile_pool(name="ps_s", bufs=2, space="PSUM") as ps_s,
            tc.tile_pool(name="ps_o", bufs=3, space="PSUM") as ps_o,
            tc.tile_pool(name="ps_m", bufs=1, space="PSUM") as ps_m,
        ):
            k_t = qkv.tile([D, B, S], F16)
            q_t = qkv.tile([D, B, NBLK, HLOC, BLK], F16)
            v_t = qkv.tile([BLK, B, NBLK, D], BF16)
            onehot_t = constp.tile([BLK, NBLK, NBLK], BF16)
            tri_t = constp.tile([BLK, HLOC, BLK], BF16)
            ebias_t = constp.tile([BLK, nbias], F32)

            # Input DMAs in per-tj consumption order, issue-parallelized
            # across four engine queues so the early units' inputs land first.
            nc.sync.dma_start(out=q_t[:, 0, 0], in_=q_d[0, :, 0])
            nc.gpsimd.dma_start(out=k_t[:, 0, 0:2 * BLK], in_=k_d[0, :, 0:2 * BLK])
            nc.scalar.dma_start(
                out=v_t[:, 0, 0:2],
                in_=v_d[0, 0:2].rearrange("si p d -> p si d"),
            )
            nc.sync.dma_start(out=q_t[:, 0, 1], in_=q_d[0, :, 1])
            nc.gpsimd.dma_start(out=onehot_t, in_=onehot_d)
            nc.scalar.dma_start(out=ebias_t, in_=ebias_d)
            nc.sync.dma_start(out=q_t[:, 0, 2:4], in_=q_d[0, :, 2:4])
            nc.gpsimd.dma_start(out=k_t[:, 0, 2 * BLK:4 * BLK],
                                in_=k_d[0, :, 2 * BLK:4 * BLK])
            nc.scalar.dma_start(out=tri_t, in_=tri_d)
            nc.sync.dma_start(
                out=v_t[:, 0, 2:4],
                in_=v_d[0, 2:4].rearrange("si p d -> p si d"),
            )
            nc.gpsimd.dma_start(out=k_t[:, 0, 4 * BLK:], in_=k_d[0, :, 4 * BLK:])
            nc.sync.dma_start(out=q_t[:, 0, 4:], in_=q_d[0, :, 4:])
            nc.gpsimd.dma_start(
                out=v_t[:, 0, 4:],
                in_=v_d[0, 4:].rearrange("si p d -> p si d"),
            )
            nc.gpsimd.dma_start(out=k_t[:, 1], in_=k_d[1])
            nc.gpsimd.dma_start(
                out=v_t[:, 1], in_=v_d[1].rearrange("si p d -> p si d")
            )
            nc.sync.dma_start(out=q_t[:, 1, 0:4], in_=q_d[1, :, 0:4])
            nc.sync.dma_start(out=q_t[:, 1, 4:], in_=q_d[1, :, 4:])

            # Build the flat pair-task list (software pipeline over PE).
            tasks = []
            for b, tj in unit_order:
                blocks = all_units[b][tj]
                unit = {"b": b, "tj": tj, "n_blk": len(blocks)}
                # per-col first/last live si in execution order, for PV/sums
                # start/stop flags (cols of a later block may start fresh)
                first_of = {}
                last_of = {}
                for bi, blk in enumerate(blocks):
                    for col in range(blk["a"], blk["e"]):
                        if col not in first_of:
                            first_of[col] = bi
                        last_of[col] = bi
                for bi, blk in enumerate(blocks):
                    # split [a, e) into runs of constant (start, stop)
                    ops = []
                    ra = blk["a"]
                    rflags = None
                    for col in range(blk["a"], blk["e"] + 1):
                        flags = (first_of[col] == bi, last_of[col] == bi) \
                            if col < blk["e"] else None
                        if flags != rflags:
                            if rflags is not None and col > ra:
                                ops.append((ra, col, rflags[0], rflags[1]))
                            ra = col
                            rflags = flags
                    blk["pv_ops"] = ops
                pairs = [blocks[i:i + 2] for i in range(0, len(blocks), 2)]
                for pi, pair in enumerate(pairs):
                    tasks.append({
                        "unit": unit, "pair": pair,
                        "first": pi == 0, "last": pi == len(pairs) - 1,
                    })

            # Per-batch shared sums PSUM tile: row tj accumulates unit tj's
            # softmax denominators; DVE zeroes it once, every sums matmul
            # accumulates with start=False, and the tile is DMA'd straight
            # from PSUM at batch end.
            sm_all = [None, None]
            group_stage = {}
            dma_rr = [0]  # round-robin out-DMA issuing engine

            def emit_qk(task):
                unit = task["unit"]
                b, tj = unit["b"], unit["tj"]
                if task["first"]:
                    unit["outp"] = ps_o.tile([D, HLOC, BLK], F32, name="outp")
                    if sm_all[b] is None:
                        sm_all[b] = ps_m.tile([NBLK, HLOC, BLK], F32,
                                              name="sm_all")
                        nc.vector.memset(sm_all[b], 0.0)
                pair = task["pair"]
                st = ps_s.tile([BLK, 2, HLOC, BLK], F32, name="st")
                pt = ptp.tile([BLK, 2, HLOC, BLK], BF16, name="pt")
                for j, blk in enumerate(pair):
                    kh = k_t[:, b, blk["si"] * BLK:(blk["si"] + 1) * BLK]
                    nc.tensor.matmul(st[:, j, :, blk["a"]:blk["e"]],
                                     kh, q_t[:, b, tj, :, blk["a"]:blk["e"]],
                                     start=True, stop=True,
                                     skip_group_check=True)
                # one ACT exp per (span, cls) run; a clean pair fuses into one
                runs = []  # (j0, j1, a, e, cls) over pair halves
                for j, blk in enumerate(pair):
                    for a, e, cls in blk["spans"]:
                        cid = 0 if cls == 0 else cls_base[b] + cls
                        if (runs and runs[-1][2] == a and runs[-1][3] == e
                                and runs[-1][4] == cid
                                and runs[-1][1] == j):
                            runs[-1] = (runs[-1][0], j + 1, a, e, cid)
                        else:
                            runs.append((j, j + 1, a, e, cid))
                for j0, j1, a, e, cid in runs:
                    nc.scalar.activation(
                        out=pt[:, j0:j1, :, a:e], in_=st[:, j0:j1, :, a:e],
                        func=mybir.ActivationFunctionType.Exp,
                        bias=ebias_t[:, cid:cid + 1],
                    )
                for j, blk in enumerate(pair):
                    if blk["diag"]:
                        # zero s > t for every head via the tri keep-mask
                        nc.vector.tensor_mul(
                            out=pt[:, j], in0=pt[:, j], in1=tri_t
                        )
                task["pt"] = pt
                # pre-add equal-span pair halves on DVE now (bf16 fast path),
                # a unit ahead of the sums matmul that consumes the result —
                # the unit of pipeline slack hides the add's latency
                if (len(pair) == 2 and pair[0]["a"] == pair[1]["a"]
                        and pair[0]["e"] == pair[1]["e"]):
                    a, e = pair[0]["a"], pair[0]["e"]
                    ps = ptsp.tile([BLK, HLOC, BLK], BF16, name="ptsum")
                    nc.vector.tensor_add(
                        out=ps[:, :, a:e],
                        in0=pt[:, 0, :, a:e], in1=pt[:, 1, :, a:e],
                    )
                    task["ptsum"] = (ps, a, e)
                else:
                    task["ptsum"] = None

            def emit_pv_unit(utasks):
                unit = utasks[0]["unit"]
                b, tj = unit["b"], unit["tj"]
                batch_last_unit = (b, tj) in ((0, 7), (1, 0))
                # all sums matmuls of the unit back-to-back first: they share
                # the one-hot weights (skips most of the LDWEIGHTS cost), and
                # the sums staging then overlaps the PV/CAST chain. One
                # matmul per block over its full live span (sums only
                # accumulate, so the pv_op start/stop splits don't apply).
                sum_ops = []
                for task in utasks:
                    if task["ptsum"] is not None:
                        ps, a, e = task["ptsum"]
                        sum_ops.append((ps, a, e))
                    else:
                        for j, blk in enumerate(task["pair"]):
                            sum_ops.append(
                                (task["pt"][:, j], blk["a"], blk["e"]))
                for oi, (src, a, e) in enumerate(sum_ops):
                    nc.tensor.matmul(
                        sm_all[b][:, :, a:e], onehot_t[:, tj],
                        src[:, :, a:e],
                        start=False,
                        stop=batch_last_unit and oi == len(sum_ops) - 1,
                        skip_group_check=True,
                    )
                for task in utasks:
                    for j, blk in enumerate(task["pair"]):
                        si = blk["si"]
                        for a, e, fst, lst in blk["pv_ops"]:
                            nc.tensor.matmul(
                                unit["outp"][:, :, a:e], v_t[:, b, si],
                                task["pt"][:, j, :, a:e],
                                start=fst, stop=lst, skip_group_check=True,
                            )
                if batch_last_unit:
                    s_sb = sstagep.tile([NBLK, HLOC, BLK], F32,
                                        name="sstage")
                    nc.vector.tensor_copy(out=s_sb, in_=sm_all[b])
                    nc.gpsimd.dma_start(out=sums_d[b], in_=s_sb)
                group, slot = unit_group[(b, tj)]
                gkey = id(group)
                if gkey not in group_stage:
                    group_stage[gkey] = stagep.tile(
                        [D, 2, HLOC, BLK], BF16, name="ostage"
                    )[:, :len(group)]
                o_sb = group_stage[gkey]
                nc.vector.tensor_copy(out=o_sb[:, slot], in_=unit["outp"])
                if slot == len(group) - 1:
                    tj0 = group[0][1]
                    eng = nc.sync if dma_rr[0] % 2 == 0 else nc.gpsimd
                    dma_rr[0] += 1
                    eng.dma_start(
                        out=outT_d[b, :, tj0:tj0 + len(group)], in_=o_sb
                    )

            # unit-level software pipeline with a 2-unit lookahead: PE always
            # has the next units' QK queued while earlier exps complete
            unit_seq = []
            for task in tasks:
                if task["first"]:
                    unit_seq.append([])
                unit_seq[-1].append(task)
            window = []
            for utasks in unit_seq:
                for task in utasks:
                    emit_qk(task)
                window.append(utasks)
                if len(window) > 2:
                    emit_pv_unit(window.pop(0))
            for utasks in window:
                emit_pv_unit(utasks)
    nc.compile()
    return nc


def kernel(query, key, value, decoder_segment_ids):
    from concourse.bass_utils import run_bass_kernel_spmd

    query = np.asarray(query, dtype=np.float32)
    key = np.asarray(key, dtype=np.float32)
    value = np.asarray(value, dtype=np.float32)
    seg = np.asarray(decoder_segment_ids, dtype=np.int32)

    structs = [_segment_structure(seg[b]) for b in range(B)]
    all_units = [s[0] for s in structs]
    all_classes = [s[1] for s in structs]
    sig = tuple(
        tuple(tuple((blk["si"], blk["a"], blk["e"], tuple(blk["spans"]),
                     blk["diag"]) for blk in blocks)
              for blocks in units)
        for units in all_units
    ) + tuple(c.tobytes() for cl in all_classes for c in cl)
    nc = _compiled_cache.get(sig)
    if nc is None:
        nc = _build_program(all_units, all_classes)
        _compiled_cache[sig] = nc

    onehot = np.zeros((BLK, NBLK, NBLK), dtype=ml_dtypes.bfloat16)
    for tj in range(NBLK):
        onehot[:, tj, tj] = 1.0
    s_i = np.arange(BLK)
    tri = (s_i[:, None] <= s_i[None, :]).astype(ml_dtypes.bfloat16)
    tri = np.ascontiguousarray(np.broadcast_to(tri[:, None, :], (BLK, HLOC, BLK)))

    nbias = sum(len(c) for c in all_classes)
    ebias = np.full((BLK, nbias), EXP_SHIFT, dtype=np.float32)
    i = 0
    for cl in all_classes:
        for rows in cl:
            ebias[:, i] += np.where(rows, MASK_BIAS, 0.0)
            i += 1

    in_maps = []
    for c in range(N_CORES):
        q_c = query[:, :, c * HLOC:(c + 1) * HLOC, :]  # (B, T, HLOC, D)
        # -> (B, D, NBLK, HLOC, BLK): element [b,d,tj,h,y] = q_c[b, tj*128+y, h, d]
        qT = np.ascontiguousarray(
            q_c.transpose(0, 3, 1, 2)  # (B, D, T, HLOC)
            .reshape(B, D, NBLK, BLK, HLOC)
            .transpose(0, 1, 2, 4, 3)
        ).astype(np.float16)
        kT = np.ascontiguousarray(
            key[:, :, c, :].transpose(0, 2, 1)
        ).astype(np.float16)  # (B, D, S)
        v_c = value[:, :, c, :].reshape(B, NBLK, BLK, D).astype(ml_dtypes.bfloat16)
        in_maps.append({"q": qT, "k": kT, "v": v_c, "ebias": ebias,
                        "onehot": onehot, "tri": tri})

    kwargs = {}
    if TRACE:
        kwargs = dict(trace=True, tmpdir=TRACE_DIR)
    res = run_bass_kernel_spmd(nc, in_maps, core_ids=list(range(N_CORES)), **kwargs)
    kernel.last_results = res

    out = np.empty((B, T, NQ, D), dtype=np.float32)
    for c in range(N_CORES):
        outT = res.results[c]["outT"]  # (B, D, NBLK, HLOC, BLK) bf16
        sums = res.results[c]["sums"]  # (B, NBLK, HLOC, BLK) f32
        o = outT.astype(np.float32).reshape(B, D, NBLK, HLOC, BLK)
        s = sums
        # out[b, tj*128+y, c*4+h, d] = o[b, d, tj, h, y] / s[b, tj, h, y]
        o = o.transpose(0, 2, 4, 3, 1).reshape(B, T, HLOC, D)
        s = s.transpose(0, 1, 3, 2).reshape(B, T, HLOC)
        out[:, :, c * HLOC:(c + 1) * HLOC, :] = o / s[:, :, :, None]
    return out
